# revision 8
# baseline (speedup 1.0000x reference)
"""ContextualAttention TRN2 kernel.

Full inputs -> full output. Sharding: 8 cores = 2 batches x 4 q-blocks of the
L=4096 attention-column dimension. Each core computes, for its 1024 columns q:

  S[p, q]  = sum_f wn[f, p] * pr[f, q]          (QK^T, K = 1152 = 9 x 128)
  E[p, q]  = exp(10 * (S - denom_q))             (denom_q = per-column softmax
                                                  shift; upper-bounds the column
                                                  max by Cauchy-Schwarz, so the
                                                  softmax is exact after the
                                                  1/colsum normalization)
  A[p, q]  = E * mfilt_p                         (post-softmax patch mask)
  colT[q,:] = (A^T @ xu) / colsum_q              (conv_transpose as GEMM)

Host side: unfold / normalization prep (pure index shuffles + one divide) and
the final col2im overlap-add.  wn has the pre-softmax mask and 1/denom_p
folded in on the host.
"""
import numpy as np

import concourse.bass as bass
import concourse.bacc as bacc
import concourse.mybir as mybir
from concourse import tile
from concourse.bass_utils import run_bass_kernel_spmd

F32 = mybir.dt.float32
AFT = mybir.ActivationFunctionType

B, C, H, W = 2, 128, 128, 128
RATE, BS = 2, 3                # attention rate, block size
Hr, Wr = H // RATE, W // RATE  # 64, 64
L = Hr * Wr                    # 4096
F = C * BS * BS                # 1152 contraction dim, 9 k-tiles
CK = C * 16                    # 2048 deconv output cols (kappa*128 + c)
QPC = L // 4                   # 1024 q columns per core
EPS = 1e-4
SCALE = 10.0
N_CORES = 8

_CACHE = {}


def _build_nc():
    nc = bacc.Bacc(None)
    wn_d = nc.declare_dram_parameter("wn", [F, L], F32, isOutput=False)
    prq_d = nc.declare_dram_parameter("prq", [F, QPC], F32, isOutput=False)
    xu_d = nc.declare_dram_parameter("xu", [L, CK], F32, isOutput=False)
    ndq_d = nc.declare_dram_parameter("ndq", [1, QPC], F32, isOutput=False)
    mrow_d = nc.declare_dram_parameter("mrow", [128, 32], F32, isOutput=False)
    col_d = nc.declare_dram_parameter("col", [QPC, CK], F32, isOutput=True)

    NPT = L // 128    # 32 p tiles
    NKT = F // 128    # 9 k tiles
    NQT = QPC // 128  # 8 q tiles
    NCH = CK // 512   # 4 ck chunks

    with tile.TileContext(nc) as tc:
        with (
            tc.tile_pool(name="apool", bufs=NPT) as apool,
            tc.tile_pool(name="const", bufs=1) as cpool,
            tc.tile_pool(name="rhs", bufs=1) as rhspool,
            tc.tile_pool(name="lhs", bufs=2) as lhspool,
            tc.tile_pool(name="xus", bufs=2) as xupool,
            tc.tile_pool(name="outs", bufs=2) as opool,
            tc.tile_pool(name="rows", bufs=2) as rowpool,
            tc.tile_pool(name="ps", bufs=8, space="PSUM") as pspool,
        ):
            # ---- resident loads -------------------------------------------
            rhs_sb = rhspool.tile([128, NKT * QPC], F32)       # 36 KB/part
            nc.sync.dma_start(
                rhs_sb[:].rearrange("p (k q) -> p k q", k=NKT),
                prq_d[:].rearrange("(k fi) q -> fi k q", fi=128))
            ndq_sb = rowpool.tile([1, QPC], F32, tag="row")
            nc.sync.dma_start(ndq_sb[:], ndq_d[:])
            m_sb = cpool.tile([128, 32], F32)
            nc.sync.dma_start(m_sb[:], mrow_d[:])
            onek1 = cpool.tile([1, 128], F32)
            nc.gpsimd.memset(onek1[:], 1.0)
            ones_col = cpool.tile([128, 1], F32)
            nc.gpsimd.memset(ones_col[:], 1.0)
            acc = cpool.tile([128, QPC], F32)
            nc.gpsimd.memset(acc[:], 0.0)
            r8 = cpool.tile([128, NQT], F32)

            # ---- phase A: S = wn^T @ prq, E = exp(10(S-dq)), acc += E -----
            a_tiles = []
            for pt in range(NPT):
                lhs = lhspool.tile([128, NKT * 128], F32)
                nc.sync.dma_start(
                    lhs[:].rearrange("p (k j) -> p k j", k=NKT),
                    wn_d[:, pt * 128:(pt + 1) * 128]
                    .rearrange("(k fi) j -> fi k j", fi=128))
                at = apool.tile([128, QPC], F32)
                for qc in range(QPC // 512):
                    ps = pspool.tile([128, 512], F32, tag="ps")
                    nc.tensor.matmul(
                        ps[:], onek1[:], ndq_sb[0:1, qc * 512:(qc + 1) * 512],
                        start=True, stop=False)
                    for k in range(NKT):
                        nc.tensor.matmul(
                            ps[:],
                            lhs[:, k * 128:(k + 1) * 128],
                            rhs_sb[:, k * QPC + qc * 512:k * QPC + qc * 512 + 512],
                            start=False, stop=(k == NKT - 1))
                    nc.scalar.activation(
                        at[:, qc * 512:(qc + 1) * 512], ps[:], AFT.Exp,
                        bias=0.0, scale=SCALE)
                nc.vector.tensor_add(acc[:], acc[:], at[:])
                nc.vector.tensor_scalar_mul(at[:], at[:], m_sb[:, pt:pt + 1])
                a_tiles.append(at)

            # ---- phase B: colsum -> r8[i, qt] = 1/colsum(q=qt*128+i) ------
            # out[m, 0] = sum_k acc[k, qt*128+m]: per-partition layout direct
            for qt in range(NQT):
                cs_ps = pspool.tile([128, 1], F32, tag="ps", name=f"csps{qt}")
                nc.tensor.matmul(
                    cs_ps[:], acc[:, qt * 128:(qt + 1) * 128], ones_col[:],
                    start=True, stop=True)
                nc.vector.tensor_copy(r8[:, qt:qt + 1], cs_ps[:])
            nc.vector.reciprocal(r8[:], r8[:])

            # ---- phase C: colT[q, ck] = sum_p A[p, q] xu[p, ck], scaled ---
            for ch in range(NCH):
                ps_c = [pspool.tile([128, 512], F32, tag="ps",
                                    name=f"psc{ch}_{i}")
                        for i in range(NQT)]
                for pt in range(NPT):
                    xt = xupool.tile([128, 512], F32)
                    nc.sync.dma_start(
                        xt[:], xu_d[pt * 128:(pt + 1) * 128,
                                    ch * 512:(ch + 1) * 512])
                    for qt in range(NQT):
                        nc.tensor.matmul(
                            ps_c[qt][:],
                            a_tiles[pt][:, qt * 128:(qt + 1) * 128],
                            xt[:],
                            start=(pt == 0), stop=(pt == NPT - 1))
                for qt in range(NQT):
                    ot = opool.tile([128, 512], F32)
                    nc.vector.tensor_scalar_mul(ot[:], ps_c[qt][:],
                                                r8[:, qt:qt + 1])
                    nc.sync.dma_start(
                        col_d[qt * 128:(qt + 1) * 128,
                              ch * 512:(ch + 1) * 512], ot[:])
    nc.compile()
    return nc


def _host_prep(x, mask):
    """Per-batch GEMM-ready operands (kappa-major feature layout)."""
    out = []
    for b in range(B):
        xr = x[b, :, ::RATE, ::RATE]
        xrp = np.pad(xr, ((0, 0), (1, 1), (1, 1)))
        pr = np.empty((9, C, L), np.float32)
        for di in range(3):
            for dj in range(3):
                pr[di * 3 + dj] = xrp[:, di:di + Hr, dj:dj + Wr].reshape(C, L)
        pr = pr.reshape(F, L)
        denom = np.sqrt((pr * pr).sum(0, dtype=np.float64).astype(np.float32)
                        + np.float32(F * EPS))

        mr = mask[b, :, ::RATE, ::RATE]
        mrp = np.pad(mr, ((0, 0), (1, 1), (1, 1)))
        msum = np.zeros((1, L), np.float32)
        for di in range(3):
            for dj in range(3):
                msum += mrp[:, di:di + Hr, dj:dj + Wr].reshape(1, L)
        mfilt = (msum[0] == 0.0).astype(np.float32)

        wn = (pr / denom[None, :]) * mfilt[None, :]

        xp = np.pad(x[b], ((0, 0), (1, 1), (1, 1)))
        xu = np.empty((L, 16, C), np.float32)
        for i in range(4):
            for j in range(4):
                blk = xp[:, i:i + 2 * Hr:2, j:j + 2 * Wr:2]
                xu[:, i * 4 + j, :] = blk.reshape(C, L).T
        out.append((np.ascontiguousarray(wn), pr, denom, mfilt,
                    np.ascontiguousarray(xu.reshape(L, CK))))
    return out


def _col2im(col):
    """col [L, CK] -> [C, H, W] overlap-add, /4."""
    canvas = np.zeros((C, H + 2, W + 2), np.float32)
    blk = col.reshape(Hr, Wr, 16, C)
    for i in range(4):
        for j in range(4):
            canvas[:, i:i + 2 * Hr:2, j:j + 2 * Wr:2] += \
                blk[:, :, i * 4 + j, :].transpose(2, 0, 1)
    return canvas[:, 1:1 + H, 1:1 + W] / 4.0


def kernel(x, mask):
    x = np.asarray(x, np.float32)
    mask = np.asarray(mask, np.float32)
    if "nc" not in _CACHE:
        _CACHE["nc"] = _build_nc()
    nc = _CACHE["nc"]

    prep = _host_prep(x, mask)
    in_maps = []
    for core in range(N_CORES):
        b, g = divmod(core, 4)
        wn, pr, denom, mfilt, xu = prep[b]
        q0 = g * QPC
        in_maps.append({
            "wn": wn,
            "prq": np.ascontiguousarray(pr[:, q0:q0 + QPC]),
            "xu": xu,
            "ndq": np.ascontiguousarray(-denom[None, q0:q0 + QPC]),
            "mrow": np.ascontiguousarray(mfilt.reshape(32, 128).T),
        })

    _CACHE["in_maps"] = in_maps
    res = run_bass_kernel_spmd(nc, in_maps, list(range(N_CORES)))

    out = np.empty((B, C, H, W), np.float32)
    for b in range(B):
        col = np.concatenate(
            [res.results[b * 4 + g]["col"] for g in range(4)], axis=0)
        out[b] = _col2im(col)
    return out


# revision 13
# speedup vs baseline: 2.1635x; 2.1635x over previous
"""ContextualAttention TRN2 kernel.

Full inputs -> full output. Sharding: 8 cores = 2 batches x 4 q-blocks of the
L=4096 attention-column dimension. Each core computes, for its 1024 columns q:

  S[p, q]  = sum_f wn[f, p] * pr[f, q]          (QK^T, K = 1152 = 9 x 128)
  E[p, q]  = exp(10 * (S - denom_q))             (denom_q = per-column softmax
                                                  shift; upper-bounds the column
                                                  max by Cauchy-Schwarz, so the
                                                  softmax is exact after the
                                                  1/colsum normalization)
  A[p, q]  = E * mfilt_p                         (post-softmax patch mask)
  colT[q,:] = (A^T @ xu) / colsum_q              (conv_transpose as GEMM)

Host side: unfold / normalization prep (pure index shuffles + one divide) and
the final col2im overlap-add.  wn has the pre-softmax mask and 1/denom_p
folded in on the host.
"""
import numpy as np

import concourse.bass as bass
import concourse.bacc as bacc
import concourse.mybir as mybir
from concourse import tile
from concourse.bass_utils import run_bass_kernel_spmd

F32 = mybir.dt.float32
F32R = mybir.dt.float32r   # full-rate (1 cyc/row, N>=256) reduced-mult fp32
AFT = mybir.ActivationFunctionType

B, C, H, W = 2, 128, 128, 128
RATE, BS = 2, 3                # attention rate, block size
Hr, Wr = H // RATE, W // RATE  # 64, 64
L = Hr * Wr                    # 4096
F = C * BS * BS                # 1152 contraction dim, 9 k-tiles
CK = C * 16                    # 2048 deconv output cols (kappa*128 + c)
QPC = L // 4                   # 1024 q columns per core
EPS = 1e-4
SCALE = 10.0
N_CORES = 8

_CACHE = {}


def _build_nc():
    nc = bacc.Bacc(None)
    wn_d = nc.declare_dram_parameter("wn", [F, L], F32R, isOutput=False)
    prq_d = nc.declare_dram_parameter("prq", [F, QPC], F32R, isOutput=False)
    xu_d = nc.declare_dram_parameter("xu", [L, CK], F32R, isOutput=False)
    ndq_d = nc.declare_dram_parameter("ndq", [1, QPC], F32R, isOutput=False)
    ones_d = nc.declare_dram_parameter("ones1", [1, 128], F32R, isOutput=False)
    mrow_d = nc.declare_dram_parameter("mrow", [128, 32], F32, isOutput=False)
    col_d = nc.declare_dram_parameter("col", [QPC, CK], F32, isOutput=True)

    NPT = L // 128    # 32 p tiles
    NKT = F // 128    # 9 k tiles
    NQT = QPC // 128  # 8 q tiles
    NCH = CK // 512   # 4 ck chunks

    with tile.TileContext(nc) as tc:
        with (
            tc.tile_pool(name="apool", bufs=NPT) as apool,
            tc.tile_pool(name="const", bufs=1) as cpool,
            tc.tile_pool(name="rhs", bufs=1) as rhspool,
            tc.tile_pool(name="lhs", bufs=2) as lhspool,
            tc.tile_pool(name="xus", bufs=2) as xupool,
            tc.tile_pool(name="outs", bufs=2) as opool,
            tc.tile_pool(name="rows", bufs=2) as rowpool,
            tc.tile_pool(name="ps", bufs=8, space="PSUM") as pspool,
        ):
            # ---- resident loads -------------------------------------------
            rhs_sb = rhspool.tile([128, NKT * QPC], F32R)       # 36 KB/part
            nc.sync.dma_start(
                rhs_sb[:].rearrange("p (k q) -> p k q", k=NKT),
                prq_d[:].rearrange("(k fi) q -> fi k q", fi=128))
            ndq_sb = rowpool.tile([1, QPC], F32R, tag="row")
            nc.sync.dma_start(ndq_sb[:], ndq_d[:])
            m_sb = cpool.tile([128, 32], F32)
            nc.sync.dma_start(m_sb[:], mrow_d[:])
            onek1 = cpool.tile([1, 128], F32R)
            nc.sync.dma_start(onek1[:], ones_d[:])
            ones_col = cpool.tile([128, 1], F32)
            nc.gpsimd.memset(ones_col[:], 1.0)
            acc = cpool.tile([128, QPC], F32)
            nc.gpsimd.memset(acc[:], 0.0)
            r8 = cpool.tile([128, NQT], F32)

            # ---- phase A: S = wn^T @ prq, E = exp(10(S-dq)), acc += E -----
            a_tiles = []
            for pt in range(NPT):
                lhs = lhspool.tile([128, NKT * 128], F32R)
                nc.sync.dma_start(
                    lhs[:].rearrange("p (k j) -> p k j", k=NKT),
                    wn_d[:, pt * 128:(pt + 1) * 128]
                    .rearrange("(k fi) j -> fi k j", fi=128))
                at = apool.tile([128, QPC], F32R)
                for qc in range(QPC // 512):
                    ps = pspool.tile([128, 512], F32, tag="ps")
                    nc.tensor.matmul(
                        ps[:], onek1[:],
                        ndq_sb[0:1, qc * 512:(qc + 1) * 512],
                        start=True, stop=False)
                    for k in range(NKT):
                        nc.tensor.matmul(
                            ps[:],
                            lhs[:, k * 128:(k + 1) * 128],
                            rhs_sb[:, k * QPC + qc * 512:
                                   k * QPC + qc * 512 + 512],
                            start=False, stop=(k == NKT - 1))
                    nc.scalar.activation(
                        at[:, qc * 512:(qc + 1) * 512], ps[:], AFT.Exp,
                        bias=m_sb[:, pt:pt + 1], scale=SCALE)
                nc.vector.tensor_add(acc[:], acc[:], at[:].bitcast(F32))
                a_tiles.append(at)

            # ---- phase B: colsum -> r8[i, qt] = 1/colsum(q=qt*128+i) ------
            # out[m, 0] = sum_k acc[k, qt*128+m]: per-partition layout direct
            for qt in range(NQT):
                cs_ps = pspool.tile([128, 1], F32, tag="ps", name=f"csps{qt}")
                nc.tensor.matmul(
                    cs_ps[:], acc[:, qt * 128:(qt + 1) * 128], ones_col[:],
                    start=True, stop=True)
                nc.vector.tensor_copy(r8[:, qt:qt + 1], cs_ps[:])
            nc.vector.reciprocal(r8[:], r8[:])

            # ---- phase C: colT[q, ck] = sum_p A[p, q] xu[p, ck], scaled ---
            for ch in range(NCH):
                ps_c = [pspool.tile([128, 512], F32, tag="ps",
                                    name=f"psc{ch}_{i}")
                        for i in range(NQT)]
                for pt in range(NPT):
                    xt = xupool.tile([128, 512], F32R)
                    nc.sync.dma_start(
                        xt[:], xu_d[pt * 128:(pt + 1) * 128,
                                    ch * 512:(ch + 1) * 512])
                    for qt in range(NQT):
                        nc.tensor.matmul(
                            ps_c[qt][:],
                            a_tiles[pt][:, qt * 128:(qt + 1) * 128],
                            xt[:],
                            start=(pt == 0), stop=(pt == NPT - 1))
                for qt in range(NQT):
                    ot = opool.tile([128, 512], F32)
                    nc.vector.tensor_scalar_mul(ot[:], ps_c[qt][:],
                                                r8[:, qt:qt + 1])
                    nc.sync.dma_start(
                        col_d[qt * 128:(qt + 1) * 128,
                              ch * 512:(ch + 1) * 512], ot[:])
    nc.compile()
    return nc


def _host_prep(x, mask):
    """Per-batch GEMM-ready operands (kappa-major feature layout)."""
    out = []
    for b in range(B):
        xr = x[b, :, ::RATE, ::RATE]
        xrp = np.pad(xr, ((0, 0), (1, 1), (1, 1)))
        pr = np.empty((9, C, L), np.float32)
        for di in range(3):
            for dj in range(3):
                pr[di * 3 + dj] = xrp[:, di:di + Hr, dj:dj + Wr].reshape(C, L)
        pr = pr.reshape(F, L)
        denom = np.sqrt((pr * pr).sum(0, dtype=np.float64).astype(np.float32)
                        + np.float32(F * EPS))

        mr = mask[b, :, ::RATE, ::RATE]
        mrp = np.pad(mr, ((0, 0), (1, 1), (1, 1)))
        msum = np.zeros((1, L), np.float32)
        for di in range(3):
            for dj in range(3):
                msum += mrp[:, di:di + Hr, dj:dj + Wr].reshape(1, L)
        mfilt = (msum[0] == 0.0).astype(np.float32)

        wn = (pr / denom[None, :]) * mfilt[None, :]

        xp = np.pad(x[b], ((0, 0), (1, 1), (1, 1)))
        xu = np.empty((L, 16, C), np.float32)
        for i in range(4):
            for j in range(4):
                blk = xp[:, i:i + 2 * Hr:2, j:j + 2 * Wr:2]
                xu[:, i * 4 + j, :] = blk.reshape(C, L).T
        out.append((np.ascontiguousarray(wn), pr, denom, mfilt,
                    np.ascontiguousarray(xu.reshape(L, CK))))
    return out


def _col2im(col):
    """col [L, CK] -> [C, H, W] overlap-add, /4."""
    canvas = np.zeros((C, H + 2, W + 2), np.float32)
    blk = col.reshape(Hr, Wr, 16, C)
    for i in range(4):
        for j in range(4):
            canvas[:, i:i + 2 * Hr:2, j:j + 2 * Wr:2] += \
                blk[:, :, i * 4 + j, :].transpose(2, 0, 1)
    return canvas[:, 1:1 + H, 1:1 + W] / 4.0


def kernel(x, mask):
    x = np.asarray(x, np.float32)
    mask = np.asarray(mask, np.float32)
    if "nc" not in _CACHE:
        _CACHE["nc"] = _build_nc()
    nc = _CACHE["nc"]

    prep = _host_prep(x, mask)
    in_maps = []
    for core in range(N_CORES):
        b, g = divmod(core, 4)
        wn, pr, denom, mfilt, xu = prep[b]
        q0 = g * QPC
        in_maps.append({
            "wn": wn,
            "prq": np.ascontiguousarray(pr[:, q0:q0 + QPC]),
            "xu": xu,
            "ndq": np.ascontiguousarray(-denom[None, q0:q0 + QPC]),
            "mrow": np.ascontiguousarray(((mfilt - 1.0) * 1e4).reshape(32, 128).T),
            "ones1": np.ones((1, 128), np.float32),
        })

    _CACHE["in_maps"] = in_maps
    res = run_bass_kernel_spmd(nc, in_maps, list(range(N_CORES)))

    out = np.empty((B, C, H, W), np.float32)
    for b in range(B):
        col = np.concatenate(
            [res.results[b * 4 + g]["col"] for g in range(4)], axis=0)
        out[b] = _col2im(col)
    return out


# revision 14
# speedup vs baseline: 2.7968x; 1.2927x over previous
"""ContextualAttention TRN2 kernel.

Full inputs -> full output. Sharding: 8 cores = 2 batches x 4 q-blocks of the
L=4096 attention-column dimension. Each core computes, for its 1024 columns q:

  S[p, q]  = sum_f wn[f, p] * pr[f, q]          (QK^T, K = 1152 = 9 x 128)
  E[p, q]  = exp(10 * (S - denom_q))             (denom_q = per-column softmax
                                                  shift; upper-bounds the column
                                                  max by Cauchy-Schwarz, so the
                                                  softmax is exact after the
                                                  1/colsum normalization)
  A[p, q]  = E * mfilt_p                         (post-softmax patch mask)
  colT[q,:] = (A^T @ xu) / colsum_q              (conv_transpose as GEMM)

Host side: unfold / normalization prep (pure index shuffles + one divide) and
the final col2im overlap-add.  wn has the pre-softmax mask and 1/denom_p
folded in on the host.
"""
import numpy as np

import concourse.bass as bass
import concourse.bacc as bacc
import concourse.mybir as mybir
from concourse import tile
from concourse.bass_utils import run_bass_kernel_spmd

F32 = mybir.dt.float32
F32R = mybir.dt.float32r   # full-rate (1 cyc/row, N>=256) reduced-mult fp32
AFT = mybir.ActivationFunctionType

B, C, H, W = 2, 128, 128, 128
RATE, BS = 2, 3                # attention rate, block size
Hr, Wr = H // RATE, W // RATE  # 64, 64
L = Hr * Wr                    # 4096
F = C * BS * BS                # 1152 contraction dim, 9 k-tiles
CK = C * 16                    # 2048 deconv output cols (kappa*128 + c)
QPC = L // 4                   # 1024 q columns per core
EPS = 1e-4
SCALE = 10.0
N_CORES = 8

_CACHE = {}


def _build_nc():
    nc = bacc.Bacc(None)
    wn_d = nc.declare_dram_parameter("wn", [F, L], F32R, isOutput=False)
    prq_d = nc.declare_dram_parameter("prq", [F, QPC], F32R, isOutput=False)
    xu_d = nc.declare_dram_parameter("xu", [L, CK], F32R, isOutput=False)
    ndq_d = nc.declare_dram_parameter("ndq", [1, QPC], F32R, isOutput=False)
    ones_d = nc.declare_dram_parameter("ones1", [1, 128], F32R, isOutput=False)
    mrow_d = nc.declare_dram_parameter("mrow", [128, 32], F32, isOutput=False)
    col_d = nc.declare_dram_parameter("col", [QPC, CK], F32, isOutput=True)

    NPT = L // 128    # 32 p tiles
    NKT = F // 128    # 9 k tiles
    NQT = QPC // 128  # 8 q tiles
    NCH = CK // 512   # 4 ck chunks

    with tile.TileContext(nc) as tc:
        with (
            tc.tile_pool(name="apool", bufs=NPT) as apool,
            tc.tile_pool(name="const", bufs=1) as cpool,
            tc.tile_pool(name="rhs", bufs=1) as rhspool,
            tc.tile_pool(name="lhs", bufs=2) as lhspool,
            tc.tile_pool(name="xus", bufs=3) as xupool,
            tc.tile_pool(name="outs", bufs=2) as opool,
            tc.tile_pool(name="rows", bufs=2) as rowpool,
            tc.tile_pool(name="ps", bufs=8, space="PSUM") as pspool,
        ):
            # ---- resident loads -------------------------------------------
            rhs_sb = rhspool.tile([128, NKT * QPC], F32R)       # 36 KB/part
            nc.sync.dma_start(
                rhs_sb[:].rearrange("p (k q) -> p k q", k=NKT),
                prq_d[:].rearrange("(k fi) q -> fi k q", fi=128))
            ndq_sb = rowpool.tile([1, QPC], F32R, tag="row")
            nc.sync.dma_start(ndq_sb[:], ndq_d[:])
            m_sb = cpool.tile([128, 32], F32)
            nc.sync.dma_start(m_sb[:], mrow_d[:])
            onek1 = cpool.tile([1, 128], F32R)
            nc.sync.dma_start(onek1[:], ones_d[:])
            ones_col = cpool.tile([128, 1], F32)
            nc.gpsimd.memset(ones_col[:], 1.0)
            acc = cpool.tile([128, QPC], F32)
            nc.gpsimd.memset(acc[:], 0.0)
            r8 = cpool.tile([128, NQT], F32)

            # ---- phase A: S = wn^T @ prq, E = exp(10(S-dq)), acc += E -----
            a_tiles = []
            for pt in range(NPT):
                lhs = lhspool.tile([128, NKT * 128], F32R)
                (nc.gpsimd if pt % 2 else nc.sync).dma_start(
                    lhs[:].rearrange("p (k j) -> p k j", k=NKT),
                    wn_d[:, pt * 128:(pt + 1) * 128]
                    .rearrange("(k fi) j -> fi k j", fi=128))
                at = apool.tile([128, QPC], F32R)
                for qc in range(QPC // 512):
                    ps = pspool.tile([128, 512], F32, tag="ps")
                    nc.tensor.matmul(
                        ps[:], onek1[:],
                        ndq_sb[0:1, qc * 512:(qc + 1) * 512],
                        start=True, stop=False)
                    for k in range(NKT):
                        nc.tensor.matmul(
                            ps[:],
                            lhs[:, k * 128:(k + 1) * 128],
                            rhs_sb[:, k * QPC + qc * 512:
                                   k * QPC + qc * 512 + 512],
                            start=False, stop=(k == NKT - 1))
                    nc.scalar.activation(
                        at[:, qc * 512:(qc + 1) * 512], ps[:], AFT.Exp,
                        bias=m_sb[:, pt:pt + 1], scale=SCALE)
                nc.vector.tensor_add(acc[:], acc[:], at[:].bitcast(F32))
                a_tiles.append(at)

            # ---- phase B: colsum -> r8[i, qt] = 1/colsum(q=qt*128+i) ------
            # out[m, 0] = sum_k acc[k, qt*128+m]: per-partition layout direct
            for qt in range(NQT):
                cs_ps = pspool.tile([128, 1], F32, tag="ps", name=f"csps{qt}")
                nc.tensor.matmul(
                    cs_ps[:], acc[:, qt * 128:(qt + 1) * 128], ones_col[:],
                    start=True, stop=True)
                nc.vector.tensor_copy(r8[:, qt:qt + 1], cs_ps[:])
            nc.vector.reciprocal(r8[:], r8[:])

            # ---- phase C: colT[q, ck] = sum_p A[p, q] xu[p, ck], scaled ---
            for ch in range(NCH):
                ps_c = [pspool.tile([128, 512], F32, tag="ps",
                                    name=f"psc{ch}_{i}")
                        for i in range(NQT)]
                for pt in range(NPT):
                    xt = xupool.tile([128, 512], F32R)
                    (nc.gpsimd if pt % 2 else nc.sync).dma_start(
                        xt[:], xu_d[pt * 128:(pt + 1) * 128,
                                    ch * 512:(ch + 1) * 512])
                    for qt in range(NQT):
                        nc.tensor.matmul(
                            ps_c[qt][:],
                            a_tiles[pt][:, qt * 128:(qt + 1) * 128],
                            xt[:],
                            start=(pt == 0), stop=(pt == NPT - 1))
                for qt in range(NQT):
                    ot = opool.tile([128, 512], F32)
                    nc.vector.tensor_scalar_mul(ot[:], ps_c[qt][:],
                                                r8[:, qt:qt + 1])
                    nc.scalar.dma_start(
                        col_d[qt * 128:(qt + 1) * 128,
                              ch * 512:(ch + 1) * 512], ot[:])
    nc.compile()
    return nc


def _host_prep(x, mask):
    """Per-batch GEMM-ready operands (kappa-major feature layout)."""
    out = []
    for b in range(B):
        xr = x[b, :, ::RATE, ::RATE]
        xrp = np.pad(xr, ((0, 0), (1, 1), (1, 1)))
        pr = np.empty((9, C, L), np.float32)
        for di in range(3):
            for dj in range(3):
                pr[di * 3 + dj] = xrp[:, di:di + Hr, dj:dj + Wr].reshape(C, L)
        pr = pr.reshape(F, L)
        denom = np.sqrt((pr * pr).sum(0, dtype=np.float64).astype(np.float32)
                        + np.float32(F * EPS))

        mr = mask[b, :, ::RATE, ::RATE]
        mrp = np.pad(mr, ((0, 0), (1, 1), (1, 1)))
        msum = np.zeros((1, L), np.float32)
        for di in range(3):
            for dj in range(3):
                msum += mrp[:, di:di + Hr, dj:dj + Wr].reshape(1, L)
        mfilt = (msum[0] == 0.0).astype(np.float32)

        wn = (pr / denom[None, :]) * mfilt[None, :]

        xp = np.pad(x[b], ((0, 0), (1, 1), (1, 1)))
        xu = np.empty((L, 16, C), np.float32)
        for i in range(4):
            for j in range(4):
                blk = xp[:, i:i + 2 * Hr:2, j:j + 2 * Wr:2]
                xu[:, i * 4 + j, :] = blk.reshape(C, L).T
        out.append((np.ascontiguousarray(wn), pr, denom, mfilt,
                    np.ascontiguousarray(xu.reshape(L, CK))))
    return out


def _col2im(col):
    """col [L, CK] -> [C, H, W] overlap-add, /4."""
    canvas = np.zeros((C, H + 2, W + 2), np.float32)
    blk = col.reshape(Hr, Wr, 16, C)
    for i in range(4):
        for j in range(4):
            canvas[:, i:i + 2 * Hr:2, j:j + 2 * Wr:2] += \
                blk[:, :, i * 4 + j, :].transpose(2, 0, 1)
    return canvas[:, 1:1 + H, 1:1 + W] / 4.0


def kernel(x, mask):
    x = np.asarray(x, np.float32)
    mask = np.asarray(mask, np.float32)
    if "nc" not in _CACHE:
        _CACHE["nc"] = _build_nc()
    nc = _CACHE["nc"]

    prep = _host_prep(x, mask)
    in_maps = []
    for core in range(N_CORES):
        b, g = divmod(core, 4)
        wn, pr, denom, mfilt, xu = prep[b]
        q0 = g * QPC
        in_maps.append({
            "wn": wn,
            "prq": np.ascontiguousarray(pr[:, q0:q0 + QPC]),
            "xu": xu,
            "ndq": np.ascontiguousarray(-denom[None, q0:q0 + QPC]),
            "mrow": np.ascontiguousarray(((mfilt - 1.0) * 1e4).reshape(32, 128).T),
            "ones1": np.ones((1, 128), np.float32),
        })

    _CACHE["in_maps"] = in_maps
    res = run_bass_kernel_spmd(nc, in_maps, list(range(N_CORES)))

    out = np.empty((B, C, H, W), np.float32)
    for b in range(B):
        col = np.concatenate(
            [res.results[b * 4 + g]["col"] for g in range(4)], axis=0)
        out[b] = _col2im(col)
    return out
